# revision 48
# baseline (speedup 1.0000x reference)
"""Trainium2 Bass kernel for nn_ComplexFaberConv (gnn_message_passing).

Strategy
--------
Host algebra: the K-hop einsum collapses (sum_k s_k W[k] -> one 128x128
effective weight per real/imag), and the degree normalization factorizes as
val_e = a[dst] * b[src].  Everything reduces to a pure gather + segment-sum
over a [2N, 256] feature table (features x effective weights x src-side
degree factors; real||imag concat).

The wire (axon RPC tunnel, ~40MB/s) dominates wall time, so the table is
built ON DEVICE instead of being uploaded: each core receives only its bf16
transposed x shard (6.4MB), computes its table shard with 5 matmuls per
128-node tile, and an 8-core AllGather replicates the full bf16 table into
device DRAM.  Phase 2 is the gather + segment-sum: for each 128-node dst
tile, gather the tile's edges in 128-edge chunks (indirect DMA), build a
selection matrix sel[e, d] = (dst_slot[e] == d) with one DVE is_equal, and
accumulate psum[128 dst, 256] += sel.T @ gathered on the tensor engine.
Outputs go back as bf16.  Host un-permutes and adds the bias row.
"""
import numpy as np
import ml_dtypes

import concourse.bass as bass
import concourse.bacc as bacc
import concourse.mybir as mybir
import concourse.tile as tile
from concourse import bass_utils

K = 3
ALPHA = 0.5
EXPONENT = -0.25
NCORES = 8
P = 128
DCAT = 256          # real||imag feature width
N = 100000
TPC = -(-N // (NCORES * P))   # 98 node tiles per core
NPC = TPC * P                 # 12544 nodes per core (padded)
NPAD = NCORES * NPC

# set by tests to run CoreSim instead of hardware
_SIM = False

_prog_cache = {}
_last_info = {}


# --------------------------------------------------------------------------
# axon exec-path patch: materialize the donated zero output buffers on
# device (jnp.zeros under jit) instead of shipping ~51MB of host zeros
# through the RPC tunnel on every call.  Identical semantics otherwise.
# --------------------------------------------------------------------------

_exec_bundles = {}


def _exec_bundle(nc, n_cores):
    """Build (and cache) everything needed to run `nc` via PJRT: jit'd
    shard_map program, on-device zero-output maker, mesh/sharding."""
    key = (id(nc), n_cores)
    bundle = _exec_bundles.get(key)
    if bundle is not None:
        return bundle
    from concourse import bass2jax as b2j
    import jax
    import jax.numpy as jnp
    from jax.experimental.shard_map import shard_map
    from jax.sharding import Mesh, NamedSharding, PartitionSpec

    b2j.install_neuronx_cc_hook()
    partition_name = (
        nc.partition_id_tensor.name if nc.partition_id_tensor else None)

    in_names, out_names, out_avals = [], [], []
    for alloc in nc.m.functions[0].allocations:
        if not isinstance(alloc, mybir.MemoryLocationSet):
            continue
        name = alloc.memorylocations[0].name
        if alloc.kind == "ExternalInput":
            if name != partition_name:
                in_names.append(name)
        elif alloc.kind == "ExternalOutput":
            out_names.append(name)
            shape = tuple(alloc.tensor_shape)
            dtype = mybir.dt.np(alloc.dtype)
            out_avals.append(jax.core.ShapedArray(shape, dtype))
    n_params = len(in_names)
    n_outs = len(out_avals)
    in_avals = []
    for name in in_names:
        mls = nc.lookup_mls(name)
        in_avals.append(jax.ShapeDtypeStruct(
            tuple(mls.tensor_shape), mybir.dt.np(mls.dtype)))
    in_names.extend(out_names)
    if partition_name is not None:
        in_names.append(partition_name)
    donate = tuple(range(n_params, n_params + n_outs))

    def _body(*args):
        operands = list(args)
        if partition_name is not None:
            operands.append(b2j.partition_id_tensor())
        outs = b2j._bass_exec_p.bind(
            *operands,
            out_avals=tuple(out_avals),
            in_names=tuple(in_names),
            out_names=tuple(out_names),
            lowering_input_output_aliases=(),
            sim_require_finite=True,
            sim_require_nnan=True,
            nc=nc,
        )
        return tuple(outs)

    devices = jax.devices()[:n_cores]
    mesh = Mesh(np.asarray(devices), ("core",))
    in_specs = (PartitionSpec("core"),) * (n_params + n_outs)
    out_specs = (PartitionSpec("core"),) * len(out_names)
    sharded = jax.jit(
        shard_map(_body, mesh=mesh, in_specs=in_specs,
                  out_specs=out_specs, check_rep=False),
        donate_argnums=donate,
        keep_unused=True,
    )
    sharding = NamedSharding(mesh, PartitionSpec("core"))
    zshapes = tuple((n_cores * a.shape[0], *a.shape[1:]) for a in out_avals)
    zdtypes = tuple(a.dtype for a in out_avals)
    mkzeros = jax.jit(
        lambda: tuple(jnp.zeros(s, d) for s, d in zip(zshapes, zdtypes)),
        out_shardings=tuple(sharding for _ in out_avals))
    bundle = dict(
        in_names=in_names, out_names=out_names, out_avals=out_avals,
        in_avals=in_avals, n_params=n_params, mesh=mesh, sharding=sharding,
        sharded=sharded, mkzeros=mkzeros, devices=devices)
    _exec_bundles[key] = bundle
    return bundle


def _prewarm_bundle(nc, n_cores):
    """Trace+compile the XLA side and materialize zeros ahead of time."""
    import jax
    b = _exec_bundle(nc, n_cores)
    gshapes = [jax.ShapeDtypeStruct((n_cores * a.shape[0], *a.shape[1:]),
                                    a.dtype)
               for a in b["in_avals"]]
    zshapes = [jax.ShapeDtypeStruct((n_cores * a.shape[0], *a.shape[1:]),
                                    a.dtype)
               for a in b["out_avals"]]
    b["sharded"].lower(*gshapes, *zshapes).compile()
    b["zeros"] = b["mkzeros"]()  # on-device, consumed by the first run


def _install_zeros_patch():
    from concourse import bass2jax as b2j
    if getattr(b2j, "_zeros_on_device", False):
        return
    orig = b2j.run_bass_via_pjrt

    def patched(nc, in_maps, n_cores):
        if n_cores == 1 or nc.dbg_addr is not None:
            return orig(nc, in_maps, n_cores)
        import time as _time
        import concurrent.futures as _cf
        b = _exec_bundle(nc, n_cores)
        _tt = {"start": _time.time()}
        concat_in = []
        for name in b["in_names"][:b["n_params"]]:
            g = in_maps[0].get("__global_" + name)
            if g is not None:
                concat_in.append(g)
            else:
                concat_in.append(np.concatenate(
                    [np.asarray(m[name]) for m in in_maps], axis=0))
        _tt["concat"] = _time.time()
        zeros = b.pop("zeros", None)
        if zeros is None:
            zeros = b["mkzeros"]()
        _tt["zeros"] = _time.time()
        out_arrs = b["sharded"](*concat_in, *zeros)
        _tt["dispatch"] = _time.time()
        for a in out_arrs:
            a.block_until_ready()
        _tt["execute"] = _time.time()
        # fetch the 8 per-device shards concurrently (the serial global
        # np.asarray path runs ~25MB/s; parallel shard fetch saturates the
        # tunnel)
        out_avals, out_names = b["out_avals"], b["out_names"]

        def fetch(shard):
            return np.asarray(shard.data)

        host_shards = []
        with _cf.ThreadPoolExecutor(max_workers=n_cores) as ex:
            for a in out_arrs:
                shards = sorted(a.addressable_shards,
                                key=lambda s: s.index[0].start or 0)
                host_shards.append(list(ex.map(fetch, shards)))
        _tt["download"] = _time.time()
        b2j._last_phases = _tt
        return [
            {name: host_shards[i][c] for i, name in enumerate(out_names)}
            for c in range(n_cores)
        ]

    b2j.run_bass_via_pjrt = patched
    b2j._zeros_on_device = True


# --------------------------------------------------------------------------
# host-side preparation
# --------------------------------------------------------------------------

def _host_prep(x_real, x_imag, W_real, W_imag, b_real, b_imag, edge_index):
    n = x_real.shape[0]
    assert n == N
    row = edge_index[0].astype(np.int32)
    col = edge_index[1].astype(np.int32)
    tpc = TPC
    nbins = NCORES * tpc

    deg_out = np.bincount(row, minlength=n).astype(np.float32)
    deg_in = np.bincount(col, minlength=n).astype(np.float32)
    with np.errstate(divide="ignore"):
        afull = np.where(deg_out > 0, deg_out ** np.float32(EXPONENT), 0.0)
        bfull = np.where(deg_in > 0, deg_in ** np.float32(EXPONENT), 0.0)
    afull = afull.astype(np.float32)
    bfull = bfull.astype(np.float32)

    s = (0.5 ** np.arange(K)).astype(np.float32)
    Wr = np.einsum("kod,k->od", W_real, s).astype(np.float32)
    Wi = np.einsum("kod,k->od", W_imag, s).astype(np.float32)
    c1 = (s @ b_real - s @ b_imag).astype(np.float32)
    c2 = (s @ b_real + s @ b_imag).astype(np.float32)

    # device weights: [d, o] layout (matmul rhs), pre-scaled, [wA|wB|wC]
    wg = np.concatenate([0.5 * Wr.T, -0.5 * Wi.T, Wi.T],
                        axis=1).astype(ml_dtypes.bfloat16)

    # phase-1 src-side scale vectors, natural node order: [P, TPC] per core
    apad = np.zeros(NPAD, dtype=np.float32)
    bpad = np.zeros(NPAD, dtype=np.float32)
    apad[:n] = afull
    bpad[:n] = bfull
    avec = apad.reshape(NCORES, TPC, P).transpose(0, 2, 1).copy()
    bvec = bpad.reshape(NCORES, TPC, P).transpose(0, 2, 1).copy()

    # ---- balance nodes into (core, tile) bins: sorted round-robin on degree
    load = deg_out + deg_in
    order = np.argsort(-load, kind="stable")
    idx = np.arange(n, dtype=np.int32)
    node_bin = np.empty(n, dtype=np.int32)
    node_slot = np.empty(n, dtype=np.int32)
    node_bin[order] = idx % nbins
    node_slot[order] = idx // nbins
    gslot = (node_bin // tpc) * NPC + (node_bin % tpc) * P + node_slot
    core_of = node_bin // tpc
    tile_of = node_bin % tpc

    fwd_cnt = np.bincount(node_bin[row], minlength=nbins)
    bwd_cnt = np.bincount(node_bin[col], minlength=nbins)
    cf = int(-(-fwd_cnt.max() // P))
    cb = int(-(-bwd_cnt.max() // P))
    cpt = cf + cb
    nch = tpc * cpt

    # packed per-edge metadata: low 18 bits = table row, high bits = dst slot
    # (slot 255 on padding lanes never matches the 0..127 iota)
    packed_all = np.full((NCORES, P, nch), 255 << 18, dtype=np.int32)
    for direction in range(2):
        dst = row if direction == 0 else col
        src = col if direction == 0 else row
        tabrow = (src // NPC) * (2 * NPC) + (src % NPC) + (0 if direction == 0 else NPC)
        dbin = node_bin[dst]
        eorder = np.argsort(dbin, kind="stable")
        dbin_s = dbin[eorder]
        slot_s = node_slot[dst][eorder]
        tab_s = tabrow[eorder]
        starts = np.searchsorted(dbin_s, np.arange(nbins + 1))
        r = np.arange(dst.shape[0]) - starts[dbin_s]
        cbase = 0 if direction == 0 else cf
        colidx = (dbin_s % tpc) * cpt + cbase + r // P
        corei = dbin_s // tpc
        packed_all[corei, r % P, colidx] = (tab_s | (slot_s << 18)).astype(np.int32)

    afac = np.zeros((NCORES, P, tpc), dtype=np.float32)
    bfac = np.zeros((NCORES, P, tpc), dtype=np.float32)
    afac[core_of, node_slot, tile_of] = afull
    bfac[core_of, node_slot, tile_of] = bfull

    # [avec | bvec | afac | bfac] per core
    fac = np.concatenate(
        [avec, bvec, afac, bfac], axis=2).astype(np.float32)

    # [packed edge meta | iota] per core
    iota = np.broadcast_to(np.arange(P, dtype=np.int32), (NCORES, P, P))
    srcpi = np.concatenate([packed_all, iota], axis=2).astype(np.int32)

    return dict(wg=wg, fac=fac, srcpi=srcpi,
                c1=c1, c2=c2, gslot=gslot, cf=cf, cb=cb, tpc=tpc, n=n)


def _make_xall(x_real, x_imag, n):
    # transposed, padded, bf16 x shards: per core [128 feat, xr|xi nodes]
    xall = np.zeros((NCORES, P, 2 * NPC), dtype=ml_dtypes.bfloat16)
    for c in range(NCORES):
        lo, hi = c * NPC, min((c + 1) * NPC, n)
        xall[c, :, :hi - lo] = x_real[lo:hi].T
        xall[c, :, NPC:NPC + hi - lo] = x_imag[lo:hi].T
    return xall


# --------------------------------------------------------------------------
# device program
# --------------------------------------------------------------------------

def _build_program(cf, cb, tpc):
    cpt = cf + cb
    nch = tpc * cpt
    nc = bacc.Bacc("TRN2", target_bir_lowering=False, debug=False,
                   num_devices=NCORES)
    f32 = mybir.dt.float32
    bf16 = mybir.dt.bfloat16
    i32 = mybir.dt.int32
    xall = nc.dram_tensor("xall", [P, 2 * NPC], bf16, kind="ExternalInput").ap()
    wg = nc.dram_tensor("wg", [P, 3 * P], bf16, kind="ExternalInput").ap()
    fac = nc.dram_tensor("fac", [P, 2 * TPC + 2 * tpc], f32,
                         kind="ExternalInput").ap()
    srcpi = nc.dram_tensor("srcpi", [P, nch + P], i32, kind="ExternalInput").ap()
    out = nc.dram_tensor("out", [tpc * P, DCAT], bf16, kind="ExternalOutput").ap()

    with tile.TileContext(nc) as tc:
        with (
            tc.tile_pool(name="meta", bufs=1) as meta_tp,
            tc.tile_pool(name="gtab", bufs=4) as gtab_tp,
            tc.tile_pool(name="g", bufs=8) as g_tp,
            tc.tile_pool(name="sel", bufs=8) as sel_tp,
            tc.tile_pool(name="post", bufs=3) as post_tp,
            tc.tile_pool(name="ps1", bufs=1, space="PSUM") as ps1_tp,
            tc.tile_pool(name="ps2", bufs=2, space="PSUM") as ps2_tp,
            tc.tile_pool(name="dram", bufs=1, space="DRAM") as dram_tp,
        ):
            x_sb = meta_tp.tile([P, 2 * NPC], bf16)
            nc.sync.dma_start(out=x_sb[:], in_=xall[:])
            wg_sb = meta_tp.tile([P, 3 * P], bf16)
            nc.sync.dma_start(out=wg_sb[:], in_=wg[:])
            fac_sb = meta_tp.tile([P, 2 * TPC + 2 * tpc], f32)
            nc.sync.dma_start(out=fac_sb[:], in_=fac[:])
            srcpi_sb = meta_tp.tile([P, nch + P], i32)
            nc.sync.dma_start(out=srcpi_sb[:], in_=srcpi[:])
            wA_sb = wg_sb[:, 0 * P:1 * P]
            wB_sb = wg_sb[:, 1 * P:2 * P]
            wC_sb = wg_sb[:, 2 * P:3 * P]
            iota_sb = srcpi_sb[:, nch:nch + P]

            # unpack the edge metadata once: table row (low 18b), dst slot
            srcs_sb = meta_tp.tile([P, nch], i32)
            nc.vector.tensor_scalar(
                out=srcs_sb[:], in0=srcpi_sb[:, :nch], scalar1=(1 << 18) - 1,
                scalar2=None, op0=mybir.AluOpType.bitwise_and)
            slot_sb = meta_tp.tile([P, nch], i32)
            nc.vector.tensor_scalar(
                out=slot_sb[:], in0=srcpi_sb[:, :nch], scalar1=18,
                scalar2=None, op0=mybir.AluOpType.logical_shift_right)

            tab_local = dram_tp.tile([2 * NPC, DCAT], bf16)
            tab_full = dram_tp.tile([NCORES * 2 * NPC, DCAT], bf16)

            # ---- phase 1: build this core's table shard
            for t in range(TPC):
                xr_t = x_sb[:, t * P:(t + 1) * P]
                xi_t = x_sb[:, NPC + t * P:NPC + (t + 1) * P]
                psH = ps1_tp.tile([P, P], f32, space="PSUM", tag="psH")
                psI1 = ps1_tp.tile([P, P], f32, space="PSUM", tag="psI1")
                psI2 = ps1_tp.tile([P, P], f32, space="PSUM", tag="psI2")
                nc.tensor.matmul(out=psH[:], lhsT=xr_t, rhs=wA_sb[:],
                                 start=True, stop=False)
                nc.tensor.matmul(out=psH[:], lhsT=xi_t, rhs=wB_sb[:],
                                 start=False, stop=True)
                nc.tensor.matmul(out=psI1[:], lhsT=xr_t, rhs=wC_sb[:],
                                 start=True, stop=False)
                nc.tensor.matmul(out=psI1[:], lhsT=xi_t, rhs=wA_sb[:],
                                 start=False, stop=True)
                nc.tensor.matmul(out=psI2[:], lhsT=xi_t, rhs=wA_sb[:],
                                 start=True, stop=True)
                gf = gtab_tp.tile([P, DCAT], bf16, tag="gf")
                nc.scalar.activation(
                    out=gf[:, :P], in_=psH[:],
                    func=mybir.ActivationFunctionType.Copy,
                    scale=fac_sb[:, TPC + t:TPC + t + 1])
                nc.scalar.activation(
                    out=gf[:, P:], in_=psI1[:],
                    func=mybir.ActivationFunctionType.Copy,
                    scale=fac_sb[:, TPC + t:TPC + t + 1])
                gb = gtab_tp.tile([P, DCAT], bf16, tag="gb")
                nc.scalar.activation(
                    out=gb[:, :P], in_=psH[:],
                    func=mybir.ActivationFunctionType.Copy,
                    scale=fac_sb[:, t:t + 1])
                nc.scalar.activation(
                    out=gb[:, P:], in_=psI2[:],
                    func=mybir.ActivationFunctionType.Copy,
                    scale=fac_sb[:, t:t + 1])
                nc.sync.dma_start(out=tab_local[t * P:(t + 1) * P], in_=gf[:])
                nc.sync.dma_start(out=tab_local[NPC + t * P:NPC + (t + 1) * P],
                                  in_=gb[:])

            # ---- replicate the table across cores
            nc.gpsimd.collective_compute(
                "AllGather",
                mybir.AluOpType.bypass,
                replica_groups=[list(range(NCORES))],
                ins=[tab_local[:].opt()],
                outs=[tab_full[:].opt()],
            )

            # ---- phase 2: gather + segment-sum over balanced dst tiles
            # (indirect DMA honors ONE offset per partition, so each 128-edge
            # chunk is its own gather)
            for t in range(tpc):
                pf = ps2_tp.tile([P, DCAT], f32, space="PSUM", tag="pf")
                pb = ps2_tp.tile([P, DCAT], f32, space="PSUM", tag="pb")
                for c in range(cpt):
                    colx = t * cpt + c
                    gt = g_tp.tile([P, DCAT], bf16, tag="gt")
                    nc.gpsimd.indirect_dma_start(
                        out=gt[:], out_offset=None, in_=tab_full[:],
                        in_offset=bass.IndirectOffsetOnAxis(
                            ap=srcs_sb[:, colx:colx + 1], axis=0))
                    sel = sel_tp.tile([P, P], bf16, tag="sel")
                    nc.vector.tensor_tensor(
                        out=sel[:],
                        in0=slot_sb[:, colx:colx + 1].to_broadcast([P, P]),
                        in1=iota_sb[:],
                        op=mybir.AluOpType.is_equal)
                    tgt = pf if c < cf else pb
                    nc.tensor.matmul(
                        out=tgt[:], lhsT=sel[:], rhs=gt[:],
                        start=(c == 0 or c == cf),
                        stop=(c == cf - 1 or c == cpt - 1))
                s1 = post_tp.tile([P, DCAT], f32, tag="s1")
                nc.scalar.activation(
                    out=s1[:], in_=pf[:],
                    func=mybir.ActivationFunctionType.Copy,
                    scale=fac_sb[:, 2 * TPC + t:2 * TPC + t + 1])
                s2 = post_tp.tile([P, DCAT], f32, tag="s2")
                nc.vector.tensor_scalar_mul(
                    out=s2[:], in0=pb[:], scalar1=fac_sb[:, 2 * TPC + tpc + t:2 * TPC + tpc + t + 1])
                ot = post_tp.tile([P, DCAT], bf16, tag="ot")
                nc.vector.tensor_tensor(
                    out=ot[:], in0=s1[:], in1=s2[:], op=mybir.AluOpType.add)
                nc.sync.dma_start(out=out[t * P:(t + 1) * P], in_=ot[:])
    nc.compile()
    return nc


def _get_program(cf, cb, tpc):
    key = (cf, cb, tpc)
    if key not in _prog_cache:
        _prog_cache[key] = _build_program(cf, cb, tpc)
    return _prog_cache[key]


# Random ~6.4-edges-per-node graphs land on (cf, cb) = (7, 7) with very high
# probability, so build that program, warm the jax/axon backend, and
# trace+compile the XLA wrapper in the background while the caller is still
# doing host prep.
def _enable_jax_cache():
    try:
        import jax
        jax.config.update("jax_compilation_cache_dir", "/root/.jax_comp_cache")
        jax.config.update("jax_persistent_cache_min_compile_time_secs", 0.0)
    except Exception:
        pass


def _speculative_build():
    import time as _t
    try:
        t0 = _t.time()
        nc = _build_program(7, 7, TPC)
        _prog_cache[(7, 7, TPC)] = nc
        t1 = _t.time()
        _install_zeros_patch()
        _enable_jax_cache()
        import jax
        jax.devices()
        t2 = _t.time()
        _prewarm_bundle(nc, NCORES)
        t3 = _t.time()
        _last_info["spec_times"] = (t1 - t0, t2 - t1, t3 - t2)
    except Exception as e:
        _last_info["spec_err"] = repr(e)


import threading as _threading

_spec_thread = _threading.Thread(target=_speculative_build, daemon=True)
_spec_thread.start()


# --------------------------------------------------------------------------
# entry point
# --------------------------------------------------------------------------

def kernel(x_real, x_imag, W_real, W_imag, b_real, b_imag, edge_index):
    import time
    t0 = time.time()
    x_real = np.asarray(x_real, dtype=np.float32)
    x_imag = np.asarray(x_imag, dtype=np.float32)
    W_real = np.asarray(W_real, dtype=np.float32)
    W_imag = np.asarray(W_imag, dtype=np.float32)
    b_real = np.asarray(b_real, dtype=np.float32)
    b_imag = np.asarray(b_imag, dtype=np.float32)
    edge_index = np.asarray(edge_index)

    # build + start uploading the big x tensor while the edge metadata is
    # still being prepared on host (the RPC tunnel is the bottleneck).
    # All uploads go through ONE thread, strictly sequential — concurrent
    # device_put calls serialize pathologically through the axon client.
    xall = _make_xall(x_real, x_imag, x_real.shape[0])
    xglobal = {}
    small_ready = _threading.Event()
    small_arrs = {}

    def _uploader():
        import time as _t
        try:
            t0 = _t.time()
            import jax
            from jax.sharding import Mesh, NamedSharding, PartitionSpec
            devices = jax.devices()[:NCORES]
            mesh = Mesh(np.asarray(devices), ("core",))
            sharding = NamedSharding(mesh, PartitionSpec("core"))
            t1 = _t.time()
            arr = jax.device_put(xall.reshape(NCORES * P, 2 * NPC), sharding)
            arr.block_until_ready()
            xglobal["xall"] = arr
            t2 = _t.time()
            small_ready.wait(timeout=60)
            for name, harr in small_arrs.items():
                a = jax.device_put(harr, sharding)
                a.block_until_ready()
                xglobal[name] = a
            _last_info["upload_times"] = (t1 - t0, t2 - t1, _t.time() - t2)
        except Exception as e:
            _last_info["upload_err"] = repr(e)

    up_thread = _threading.Thread(target=_uploader, daemon=True)
    if not _SIM:
        up_thread.start()

    prep = _host_prep(x_real, x_imag, W_real, W_imag, b_real, b_imag, edge_index)
    t1 = time.time()
    tpc = prep["tpc"]

    small_arrs["wg"] = np.concatenate([prep["wg"]] * NCORES, axis=0)
    small_arrs["fac"] = prep["fac"].reshape(NCORES * P, -1)
    small_arrs["srcpi"] = prep["srcpi"].reshape(NCORES * P, -1)
    small_ready.set()

    _spec_thread.join()
    nc = _get_program(prep["cf"], prep["cb"], tpc)
    t2 = time.time()

    in_maps = []
    for corei in range(NCORES):
        in_maps.append({
            "xall": xall[corei],
            "wg": prep["wg"],
            "fac": prep["fac"][corei],
            "srcpi": prep["srcpi"][corei],
        })

    if _SIM:
        from concourse import bass_interp
        outs = []
        for corei in range(NCORES):
            sim = bass_interp.CoreSim(nc)
            for k, v in in_maps[corei].items():
                sim.tensor(k)[:] = v
            sim.simulate()
            outs.append(sim.tensor("out").copy())
    else:
        t3 = time.time()
        _install_zeros_patch()
        up_thread.join()
        for name in ("xall", "wg", "fac", "srcpi"):
            if name in xglobal:
                in_maps[0]["__global_" + name] = xglobal[name]
        res = bass_utils.run_bass_kernel_spmd(
            nc, in_maps, core_ids=list(range(NCORES)))
        _last_info["exec_wall_s"] = time.time() - t3
        _last_info["nc"] = nc
        _last_info["in_maps"] = in_maps
        outs = [r["out"] for r in res.results]
    _last_info["prep_s"] = t1 - t0
    _last_info["compile_s"] = t2 - t1

    full = np.concatenate(outs, axis=0)       # [NPAD, 256] bf16
    out_nodes = full[prep["gslot"]]           # [n, 256] bf16 (cheap gather)
    total_real = out_nodes[:, :P].astype(np.float32) + prep["c1"][None, :]
    total_imag = out_nodes[:, P:].astype(np.float32) + prep["c2"][None, :]
    return total_real, total_imag
